# revision 13
# baseline (speedup 1.0000x reference)
# Trainium2 Bass kernel for nn_BDH_66056597013022 (dense_transformer).
#
# Model (per reference):
#   v = LN(emb_w[tokens])                                  [B,T,D]
#   6x: x  = relu(v @ Dx_h)            per head            [B,H,T,Dh]
#       xr = RoPE(x)
#       a  = (xr @ xr^T) @ v  ==  xr @ (xr^T @ v)          [B,H,T,D]
#       y  = relu(a @ Dy_h) * x                            [B,H,T,Dh]
#       v  = LN(v + LN(concat_h(y) @ E))
#   out = v @ readout                                      [B,T,V]
#
# Shapes: B=4 T=1024 H=4 N=4096 D=256 L=6 V=256, Dh=N/H=1024.
#
# Sharding (8 cores): core c -> batch b=c//2, head-pair hp=c%2 (heads 2hp,2hp+1).
# Cross-core coupling is only the head-sum z = y @ E: one 2-rank bf16
# AllReduce per layer between cores {2b,2b+1}; both cores then redundantly
# compute the LN/v-update. Even cores' outputs are returned.
#
# Attention is computed reassociated: Wv = xr^T @ v  [Dh,D], a = xr @ Wv.
# Since D=256 << T=1024 this is 2.5x fewer FLOPs than scores@v. It needs
# xr in both layouts ([Dh,T] from RoPE and [T,Dh] for the Wv contraction);
# the second comes from 64 PE transposes per head, interleaved with the Wv
# matmuls (k-outer) so transposes/evictions/matmuls pipeline.
#
# Everything on the matmul path is bf16 (inputs pre-cast on host, DMA'd
# directly); the residual stream v is kept f32r for the LN chain with a
# bf16 shadow copy (v_bf/vT) for matmul operands. PSUM stays fp32.
#
# On-chip layouts per core (SBUF), partition dim first:
#   v_sb  [T,D]  8x[128,256] f32r     v_bf same bf16      vT [D,T] 2x[128,1024] bf16
#   xT,xr,xr_std,y: 8x[128,1024] bf16 each (xr_std is [T,Dh])
#   Wv [Dh,D] packed as 2x[128,1024] bf16 (4 e-blocks x 256 cols each)
#   aT [D,T] 2x[128,1024] bf16;  z [D,T] 2x[128,1024] bf16
# PSUM: psX 2x(2 banks) rotating x/tp/aT/y + boundary, psW 2x(1 bank) Wv
# (two passes of 4 e-blocks), psZ 2x(1 bank) z quarters (n-outer z loop).

import os
import numpy as np

B, T, H, N, D, L, V = 4, 1024, 4, 4096, 256, 6, 256
Dh = N // H
EPS = 1e-5
NCORES = 8
P = 128
NT = T // P   # 8 token tiles
ND = D // P   # 2 model-dim tiles
NDh = Dh // P  # 8 head-dim tiles

_CACHE = {}
LAST_RESULT = None


def _build_program():
    from contextlib import ExitStack

    import concourse.bass as bass
    import concourse.bacc as bacc
    import concourse.tile as tile
    import concourse.mybir as mybir
    from concourse.masks import make_identity

    f32 = mybir.dt.float32
    f32r = mybir.dt.float32r
    bf16 = mybir.dt.bfloat16
    AF = mybir.ActivationFunctionType
    ALU = mybir.AluOpType
    ts = bass.ts

    DEBUG = bool(int(os.environ.get("KERNEL_DEBUG", "0")))
    nc = bacc.Bacc("TRN2", target_bir_lowering=False, debug=False,
                   enable_asserts=False, num_devices=NCORES)

    d_oh = nc.dram_tensor("onehotT", [V, T], bf16, kind="ExternalInput").ap()
    d_ew = nc.dram_tensor("emb_w", [V, D], bf16, kind="ExternalInput").ap()
    d_cos = nc.dram_tensor("cosT", [Dh // 2, T], bf16, kind="ExternalInput").ap()
    d_sin = nc.dram_tensor("sinT", [Dh // 2, T], bf16, kind="ExternalInput").ap()
    d_dx = nc.dram_tensor("dx", [2 * D, Dh], bf16, kind="ExternalInput").ap()
    d_dy = nc.dram_tensor("dy", [2 * D, Dh], bf16, kind="ExternalInput").ap()
    d_eh = nc.dram_tensor("eh", [2 * Dh, D], bf16, kind="ExternalInput").ap()
    d_ro = nc.dram_tensor("readout", [D, V], bf16, kind="ExternalInput").ap()
    d_out = nc.dram_tensor("out", [T, V], f32, kind="ExternalOutput").ap()
    d_dbg = {}
    if DEBUG:
        for nm, shp in [("v0", [T, D]), ("xr00", [P, T]), ("xrs0", [P, Dh]),
                        ("wv0", [P, T]), ("aT00", [P, T]), ("y00", [P, 512]),
                        ("z0", [P, T]), ("zq0", [P, T]), ("v1", [T, D])]:
            d_dbg[nm] = nc.dram_tensor(
                f"dbg_{nm}", shp, f32, kind="ExternalOutput").ap()

    with tile.TileContext(nc) as tc, ExitStack() as ctx:
        wpool = ctx.enter_context(tc.tile_pool(name="weights", bufs=1))
        vpool = ctx.enter_context(tc.tile_pool(name="vpool", bufs=1))
        # xT/xr: 8 tiles per head and both heads' tiles are alive at once
        # (head1's allocated while head0's still feed aT/fusion) -> 16 deep
        xpool = ctx.enter_context(tc.tile_pool(name="xpool", bufs=16))
        xrpool = ctx.enter_context(tc.tile_pool(name="xrpool", bufs=16))
        xspool = ctx.enter_context(tc.tile_pool(name="xspool", bufs=8))
        wvpool = ctx.enter_context(tc.tile_pool(name="wvpool", bufs=2))
        apool = ctx.enter_context(tc.tile_pool(name="apool", bufs=2))
        ypool = ctx.enter_context(tc.tile_pool(name="ypool", bufs=4))
        zpool = ctx.enter_context(tc.tile_pool(name="zpool", bufs=1))
        zqpool = ctx.enter_context(tc.tile_pool(name="zqpool", bufs=1))
        lnpool = ctx.enter_context(tc.tile_pool(name="lnpool", bufs=3))
        stpool = ctx.enter_context(tc.tile_pool(name="stpool", bufs=2))
        # PSUM budget (8 banks): psX 2x2 + psW 1x2 + psZ 2x1 = 8
        psX = ctx.enter_context(tc.tile_pool(name="psX", bufs=2, space="PSUM"))
        psW = ctx.enter_context(tc.tile_pool(name="psW", bufs=1, space="PSUM"))
        psZ = ctx.enter_context(tc.tile_pool(name="psZ", bufs=2, space="PSUM"))
        dpool = ctx.enter_context(tc.tile_pool(name="drampool", bufs=2, space="DRAM"))

        # ---- persistent weights: direct bf16 DMA, no round-copies ----
        oh_sb = []
        for k in range(ND):
            t_ = wpool.tile([P, T], bf16, tag=f"oh{k}", name=f"oh{k}")
            nc.sync.dma_start(t_[:], d_oh[ts(k, P), :])
            oh_sb.append(t_)
        ew_sb = []
        for k in range(ND):
            t_ = wpool.tile([P, D], bf16, tag=f"ew{k}", name=f"ew{k}")
            nc.sync.dma_start(t_[:], d_ew[ts(k, P), :])
            ew_sb.append(t_)
        cos_sb = []
        sin_sb = []
        for i in range(4):
            t_ = wpool.tile([P, T], bf16, tag=f"cos{i}", name=f"cos{i}")
            nc.sync.dma_start(t_[:], d_cos[ts(i, P), :])
            cos_sb.append(t_)
        for i in range(4):
            t_ = wpool.tile([P, T], bf16, tag=f"sin{i}", name=f"sin{i}")
            nc.sync.dma_start(t_[:], d_sin[ts(i, P), :])
            sin_sb.append(t_)
        dx_sb = []
        for i in range(4):
            t_ = wpool.tile([P, Dh], bf16, tag=f"dx{i}", name=f"dx{i}")
            nc.sync.dma_start(t_[:], d_dx[ts(i, P), :])
            dx_sb.append(t_)
        dy_sb = []
        for i in range(4):
            t_ = wpool.tile([P, Dh], bf16, tag=f"dy{i}", name=f"dy{i}")
            nc.sync.dma_start(t_[:], d_dy[ts(i, P), :])
            dy_sb.append(t_)
        eh_sb = []  # eh_sb[j][k]: E rows for local head j, e-block k
        for j in range(2):
            row = []
            for k in range(NDh):
                t_ = wpool.tile([P, D], bf16, tag=f"eh{j}_{k}", name=f"eh{j}_{k}")
                nc.sync.dma_start(t_[:], d_eh[ts(8 * j + k, P), :])
                row.append(t_)
            eh_sb.append(row)
        ro_sb = []
        for k in range(ND):
            t_ = wpool.tile([P, V], bf16, tag=f"ro{k}", name=f"ro{k}")
            nc.sync.dma_start(t_[:], d_ro[ts(k, P), :])
            ro_sb.append(t_)

        ident = wpool.tile([P, P], f32, tag="ident", name="ident")
        make_identity(nc, ident)
        identb = wpool.tile([P, P], bf16, tag="identb", name="identb")
        nc.scalar.copy(identb[:], ident[:])
        epsc = wpool.tile([P, 1], f32, tag="epsc", name="epsc")
        nc.gpsimd.memset(epsc[:], EPS)
        warmsink = wpool.tile([P, 1], f32, tag="warmsink", name="warmsink")

        def warm(n_mms, label, width=512):
            # HAM re-throttles the PE to 1.2 GHz after ~3.4us idle; feed it
            # dependency-free matmuls during known stall windows.
            wps = psX.tile([P, width], f32, tag="psX", name=f"warm_{label}")
            for i in range(n_mms):
                nc.tensor.matmul(wps[:], dx_sb[0][:, 0:P], dx_sb[1][:, 0:width],
                                 start=(i == 0), stop=(i == n_mms - 1))
            nc.scalar.copy(warmsink[:], wps[:, 0:1])

        def warm_on(dep_ap, label, n_mms=2, width=256):
            # warm matmuls that WAIT on dep_ap: naturally spaced through a
            # stall window by the producer chain of dep_ap.
            wps = psX.tile([P, width], f32, tag="psX", name=f"warmd_{label}")
            for i in range(n_mms):
                nc.tensor.matmul(wps[:], dx_sb[0][:, 0:P], dep_ap,
                                 start=(i == 0), stop=(i == n_mms - 1))
            nc.scalar.copy(warmsink[:], wps[:, 0:1])

        # ---- persistent activations ----
        v_sb = [vpool.tile([P, D], f32, tag=f"v{m}", name=f"v{m}")
                for m in range(NT)]
        v_bf = [vpool.tile([P, D], bf16, tag=f"vb{m}", name=f"vb{m}")
                for m in range(NT)]
        vT_sb = [vpool.tile([P, T], bf16, tag=f"vT{k}", name=f"vT{k}")
                 for k in range(ND)]

        def ln_stats(src_ap):
            st6 = stpool.tile([P, 6], f32, tag="st6", name="st6")
            nc.vector.bn_stats(st6[:], src_ap)
            mv = stpool.tile([P, 2], f32, tag="mv", name="mv")
            nc.vector.bn_aggr(mv[:], st6[:])
            sd = stpool.tile([P, 1], f32, tag="sd", name="sd")
            nc.scalar.activation(sd[:], mv[:, 1:2], AF.Sqrt, bias=epsc[:], scale=1.0)
            rstd = stpool.tile([P, 1], f32, tag="rstd", name="rstd")
            nc.vector.reciprocal(rstd[:], sd[:])
            nmr = stpool.tile([P, 1], f32, tag="nmr", name="nmr")
            nc.vector.scalar_tensor_tensor(
                nmr[:], mv[:, 0:1], -1.0, rstd[:], op0=ALU.mult, op1=ALU.mult)
            return rstd, nmr

        def layer_norm(src_ap, dst_ap):
            rstd, nmr = ln_stats(src_ap)
            nc.gpsimd.tensor_scalar(dst_ap, src_ap, rstd[:], nmr[:],
                                    op0=ALU.mult, op1=ALU.add)

        def layer_norm_dual(src_ap, m):
            # v_sb[m] (f32) and v_bf[m] (bf16) from one stats pass; applies
            # on gpsimd (otherwise ACT backs up and stalls x-phase evictions)
            rstd, nmr = ln_stats(src_ap)
            nc.gpsimd.tensor_scalar(v_sb[m][:], src_ap, rstd[:], nmr[:],
                                    op0=ALU.mult, op1=ALU.add)
            nc.gpsimd.tensor_scalar(v_bf[m][:], src_ap, rstd[:], nmr[:],
                                    op0=ALU.mult, op1=ALU.add)

        def dump(dst_ap, src_ap, rows=P, name="dbg"):
            stg = lnpool.tile([P, src_ap.free_size()], f32, tag="dbgstg",
                              name=f"stg_{name}")
            nc.scalar.copy(stg[:], src_ap)
            nc.sync.dma_start(dst_ap, stg[:])

        def transpose_v(spaced_warm=False):
            # vT[d,t] <- v_bf[t,d]; 8 transposes assemble one [128,1024] bank
            for kd in range(ND):
                tps = psX.tile([P, T], bf16, tag="psX", name=f"vtp{kd}")
                for m in range(NT):
                    nc.tensor.transpose(
                        tps[:, ts(m, P)], v_bf[m][:, ts(kd, P)], identb[:])
                nc.vector.tensor_copy(vT_sb[kd][:], tps[:])

        # ---- embedding: v0 = LN(onehot @ emb_w) ----
        for m in range(NT):
            eps_t = psX.tile([P, D], f32, tag="psX", name="embps")
            for k in range(ND):
                nc.tensor.matmul(eps_t[:], oh_sb[k][:, ts(m, P)], ew_sb[k][:],
                                 start=(k == 0), stop=(k == ND - 1))
            emb_t = lnpool.tile([P, D], f32, tag="w", name="embt")
            nc.scalar.copy(emb_t[:], eps_t[:])
            layer_norm_dual(emb_t[:], m)
            if DEBUG:
                dump(d_dbg["v0"][ts(m, P), :], v_sb[m][:],
                     name=f"v0_{m}")
        transpose_v()

        rg = [[0, 1], [2, 3], [4, 5], [6, 7]]

        for layer in range(L):
            z_sb = [zpool.tile([P, T], bf16, tag=f"z{i}", name=f"z{i}_{layer}")
                    for i in range(2)]
            zin = dpool.tile([2 * P, T], bf16, tag="zin", name=f"zin_{layer}")
            zout = dpool.tile([2 * P, T], bf16, tag="zout",
                              name=f"zout_{layer}")
            xT = [[None] * NDh, [None] * NDh]
            xr = [[None] * NDh, [None] * NDh]

            # ---- A: x = relu(Dx^T @ vT) both heads, RoPE on DVE trailing ----
            for j in range(2):
                for mp in range(4):
                    pair = []
                    for mm in (mp, mp + 4):
                        xps = psX.tile([P, T], f32, tag="psX", name="xps")
                        for n in range(2):
                            for k in range(ND):
                                nc.tensor.matmul(
                                    xps[:, ts(n, 512)],
                                    dx_sb[2 * j + k][:, ts(mm, P)],
                                    vT_sb[k][:, ts(n, 512)],
                                    start=(k == 0), stop=(k == ND - 1))
                        xt = xpool.tile([P, T], bf16, tag="xT", name=f"xT{j}_{mm}")
                        nc.scalar.activation(xt[:], xps[:], AF.Relu)
                        pair.append(xt)
                        xT[j][mm] = xt
                    if j == 0:
                        # head0 RoPE immediately (its xr gates the tp phase)
                        emit_rope(nc, xrpool, stpool, cos_sb, sin_sb,
                                  pair, xr[j], mp, j, layer)
            # head1 RoPE is emitted inside head0's tp/Wv/aT phase below so the
            # DVE order is: rope0, tpe0(k), rope1, fusion0, tpe1, fusion1.
            if DEBUG and layer == 0:
                dump(d_dbg["xr00"][:], xr[0][0][:], name="xr00")

            aT = [[None, None], [None, None]]
            for j in range(2):
                # ---- B/C: xr_std = tp(xr) interleaved k-outer with
                #           Wv = xr^T @ v  (psW pass1 e-blocks 0-3, pass2 4-7)
                xrs = [None] * NT
                wv_sb = [None, None]
                for half in range(2):
                    wvp = psW.tile([P, T], f32, tag="psW", name=f"wv{half}")
                    # eb-outer: each e-block's 8-mm accumulation chain is
                    # contiguous. Interleaving chains that share a PSUM bank
                    # is WRONG: start=True clears has_written for the WHOLE
                    # bank, so a sibling chain's k=0 contribution gets
                    # overwritten at its k=1. Transposes (non-accumulating,
                    # other banks) interleave freely with the first chain.
                    for eb in range(4):
                        e = 4 * half + eb
                        for k in range(NT):
                            if half == 0 and eb == 0:
                                tpp = psX.tile([P, Dh], bf16, tag="psX",
                                               name=f"tpp{k}")
                                for kk in range(NDh):
                                    nc.tensor.transpose(
                                        tpp[:, ts(kk, P)],
                                        xr[j][kk][:, ts(k, P)], identb[:])
                                xs = xspool.tile([P, Dh], bf16, tag="xrs",
                                                 name=f"xrs{k}")
                                nc.vector.tensor_copy(xs[:], tpp[:])
                                xrs[k] = xs
                                if DEBUG and layer == 0 and j == 0 and k == 0:
                                    dump(d_dbg["xrs0"][:], xs[:], name="xrs0")
                            nc.tensor.matmul(
                                wvp[:, ts(eb, D)],
                                xrs[k][:, ts(e, P)],
                                v_bf[k][:],
                                start=(k == 0), stop=(k == NT - 1))
                    wvt = wvpool.tile([P, T], bf16, tag="wv", name=f"wv{half}")
                    nc.scalar.copy(wvt[:], wvp[:])
                    wv_sb[half] = wvt
                    if DEBUG and layer == 0 and j == 0 and half == 0:
                        dump(d_dbg["wv0"][:], wvt[:], name="wv0")
                    if j == 0 and half == 0:
                        # head1 RoPE here: after tpe0 ops, before fusion0
                        for mp in range(4):
                            emit_rope(nc, xrpool, stpool, cos_sb, sin_sb,
                                      [xT[1][mp], xT[1][mp + 4]], xr[1],
                                      mp, 1, layer)

                # ---- D: aT = (xr @ Wv)^T = Wv^T-as-lhsT @ xr ----
                for m in range(ND):
                    atp = psX.tile([P, T], f32, tag="psX", name=f"atp{m}")
                    for n in range(2):
                        for k in range(NDh):
                            nc.tensor.matmul(
                                atp[:, ts(n, 512)],
                                wv_sb[k // 4][:, (k % 4) * D + m * P:
                                              (k % 4) * D + (m + 1) * P],
                                xr[j][k][:, ts(n, 512)],
                                start=(k == 0), stop=(k == NDh - 1))
                    at = apool.tile([P, T], bf16, tag="aT", name=f"aT{m}")
                    nc.scalar.copy(at[:], atp[:])
                    aT[j][m] = at
                    if DEBUG and layer == 0 and j == 0 and m == 0:
                        dump(d_dbg["aT00"][:], at[:], name="aT00")

                # ---- E: y = relu(Dy^T @ aT) * x ; z += E_h^T @ y  (n-outer,
                #          z quarters in psZ so only 2 banks persist) ----
                for n in range(2):
                    z_ps = [psZ.tile([P, 512], f32, tag="psZ", name=f"zps{i}")
                            for i in range(2)]
                    y_half = [None] * NDh

                    def emit_zn(k):
                        for m in range(ND):
                            nc.tensor.matmul(
                                z_ps[m][:],
                                eh_sb[j][k][:, ts(m, P)],
                                y_half[k][:],
                                start=(k == 0), stop=(k == NDh - 1))

                    for k in range(NDh):
                        yps = psX.tile([P, 512], f32, tag="psX", name="yps")
                        for kk in range(ND):
                            nc.tensor.matmul(
                                yps[:],
                                dy_sb[2 * j + kk][:, ts(k, P)],
                                aT[j][kk][:, ts(n, 512)],
                                start=(kk == 0), stop=(kk == ND - 1))
                        yh = ypool.tile([P, 512], bf16, tag="y", name=f"y{k}")
                        nc.vector.scalar_tensor_tensor(
                            yh[:], yps[:], 0.0, xT[j][k][:, ts(n, 512)],
                            op0=ALU.max, op1=ALU.mult)
                        y_half[k] = yh
                        if DEBUG and layer == 0 and j == 0 and n == 0 and k == 0:
                            dump(d_dbg["y00"][:], yh[:], name="y00")
                        if k > 0:
                            emit_zn(k - 1)
                    emit_zn(NDh - 1)
                    if j == 0:
                        for i in range(2):
                            nc.scalar.copy(z_sb[i][:, ts(n, 512)], z_ps[i][:])
                    else:
                        for i in range(2):
                            nc.vector.scalar_tensor_tensor(
                                z_sb[i][:, ts(n, 512)], z_ps[i][:], 0.0,
                                z_sb[i][:, ts(n, 512)],
                                op0=ALU.add, op1=ALU.add)
                            nc.sync.dma_start(zin[ts(i, P), ts(n, 512)],
                                              z_sb[i][:, ts(n, 512)])

            if DEBUG and layer == 0:
                dump(d_dbg["z0"][:], z_sb[0][:], name="z0")
            # ---- boundary: one bf16 AllReduce of zT over the core pair ----
            nc.gpsimd.collective_compute(
                "AllReduce", mybir.AluOpType.add,
                ins=[zin.opt()], outs=[zout.opt()],
                replica_groups=rg)
            zr = [zqpool.tile([P, T], bf16, tag=f"zr{i}", name=f"zr{i}_{layer}")
                  for i in range(2)]
            for i in range(2):
                nc.sync.dma_start(zr[i][:], zout[ts(i, P), :])
            warm(84, f"ar{layer}")

            # post-AR: transpose zT -> [T,D] blocks, then LN chain per m-tile
            zq = [None, None]
            for half in range(2):
                zqp = psX.tile([P, T], bf16, tag="psX", name=f"zqp{half}")
                for mm in range(4):
                    m = 4 * half + mm
                    for kd in range(ND):
                        nc.tensor.transpose(
                            zqp[:, mm * D + kd * P: mm * D + (kd + 1) * P],
                            zr[kd][:, ts(m, P)], identb[:])
                zqt = zqpool.tile([P, T], bf16, tag=f"zq{half}",
                                  name=f"zq{half}_{layer}")
                nc.vector.tensor_copy(zqt[:], zqp[:])
                zq[half] = zqt
                if DEBUG and layer == 0 and half == 0:
                    dump(d_dbg["zq0"][:], zqt[:], name="zq0")
            warm(20, f"ln{layer}")
            for m in range(NT):
                zb = zq[m // 4][:, ts(m % 4, D)]
                u = lnpool.tile([P, D], f32, tag="u", name=f"u{m}")
                layer_norm(zb, u[:])
                w = lnpool.tile([P, D], f32, tag="w", name=f"w{m}")
                nc.vector.tensor_add(w[:], v_sb[m][:], u[:])
                layer_norm_dual(w[:], m)
                warm_on(v_bf[m][:], f"ln{layer}_{m}")
                if DEBUG and layer == 0:
                    dump(d_dbg["v1"][ts(m, P), :], v_sb[m][:],
                         name=f"v1_{m}")
            transpose_v()

        # ---- readout ----
        for m in range(NT):
            rps = psX.tile([P, V], f32, tag="psX", name="rps")
            for k in range(ND):
                nc.tensor.matmul(rps[:], vT_sb[k][:, ts(m, P)], ro_sb[k][:],
                                 start=(k == 0), stop=(k == ND - 1))
            o_sb = lnpool.tile([P, V], f32, tag="o", name=f"o{m}")
            nc.scalar.copy(o_sb[:], rps[:])
            nc.sync.dma_start(d_out[ts(m, P), :], o_sb[:])

    nc.compile()
    return nc


def emit_rope(nc, xrpool, stpool, cos_sb, sin_sb, pair, xr_out, mp, j, layer):
    # RoPE on the (mp, mp+4) tile pair, all bf16 (DVE 2x mode):
    #   xr_lo = lo*cos - hi*sin ;  xr_hi = hi*cos + lo*sin
    import concourse.mybir as mybir  # noqa: F401
    bf16 = mybir.dt.bfloat16
    P_, T_ = 128, 1024
    cm, sm = cos_sb[mp], sin_sb[mp]
    lo, hi = pair
    xrl = xrpool.tile([P_, T_], bf16, tag="xr", name=f"xr{j}_{mp}")
    xrh = xrpool.tile([P_, T_], bf16, tag="xr", name=f"xr{j}_{mp + 4}")
    t1 = stpool.tile([P_, T_], bf16, tag="ropetmp", name="rt1")
    nc.gpsimd.tensor_mul(t1[:], hi[:], sm[:])
    nc.vector.tensor_mul(xrl[:], lo[:], cm[:])
    nc.vector.tensor_sub(xrl[:], xrl[:], t1[:])
    t2 = stpool.tile([P_, T_], bf16, tag="ropetmp", name="rt2")
    nc.gpsimd.tensor_mul(t2[:], lo[:], sm[:])
    nc.vector.tensor_mul(xrh[:], hi[:], cm[:])
    nc.vector.tensor_add(xrh[:], xrh[:], t2[:])
    xr_out[mp], xr_out[mp + 4] = xrl, xrh


def _get_program():
    if "nc" not in _CACHE:
        _CACHE["nc"] = _build_program()
    return _CACHE["nc"]


def _rope_tables():
    inv = (1.0 / (10000.0 ** (np.arange(0, Dh, 2, dtype=np.float32) / Dh)))
    tt = np.arange(T, dtype=np.float32)
    freqs = np.outer(tt, inv).astype(np.float32)  # [T, Dh/2]
    cosT = np.ascontiguousarray(np.cos(freqs).T)
    sinT = np.ascontiguousarray(np.sin(freqs).T)
    return cosT, sinT


def kernel(**inputs):
    global LAST_RESULT
    import ml_dtypes
    from concourse import bass_utils

    bf = ml_dtypes.bfloat16
    tokens = np.asarray(inputs["tokens"])
    emb_w = np.ascontiguousarray(inputs["emb_w"], dtype=np.float32)
    E = np.ascontiguousarray(inputs["E"], dtype=np.float32)
    Dx = np.ascontiguousarray(inputs["Dx"], dtype=np.float32)
    Dy = np.ascontiguousarray(inputs["Dy"], dtype=np.float32)
    readout = np.ascontiguousarray(inputs["readout"], dtype=np.float32)

    cosT, sinT = _rope_tables()

    in_maps = []
    for c in range(NCORES):
        b, hp = c // 2, c % 2
        oh = np.zeros((V, T), dtype=np.float32)
        oh[np.asarray(tokens[b], dtype=np.int64), np.arange(T)] = 1.0
        in_maps.append({
            "onehotT": oh.astype(bf),
            "emb_w": emb_w.astype(bf),
            "cosT": cosT.astype(bf),
            "sinT": sinT.astype(bf),
            "dx": np.ascontiguousarray(
                Dx[2 * hp:2 * hp + 2].reshape(2 * D, Dh)).astype(bf),
            "dy": np.ascontiguousarray(
                Dy[2 * hp:2 * hp + 2].reshape(2 * D, Dh)).astype(bf),
            "eh": np.ascontiguousarray(
                E[2 * hp * Dh:(2 * hp + 2) * Dh]).astype(bf),
            "readout": readout.astype(bf),
        })

    nc = _get_program()
    res = bass_utils.run_bass_kernel_spmd(
        nc, in_maps, core_ids=list(range(NCORES)),
        trace=bool(int(os.environ.get("KERNEL_TRACE", "0"))))
    LAST_RESULT = res
    out = np.stack([res.results[2 * b]["out"] for b in range(B)], axis=0)
    return out
